# revision 20
# baseline (speedup 1.0000x reference)
"""Entmax-1.5 (alpha-entmax via bisection) Trainium2 kernel, v2.

Problem: p = entmax_bisect(where(mask, scores, -1e9), alpha=1.5) over the
last dim of a [16384, 4096] f32 tensor, data-parallel over 8 NeuronCores
(2048 rows per core).

Math: for alpha=1.5, p_i = relu(z_i - tau)^2 / f(tau) with
f(sigma) = sum relu(z - sigma)^2 and f(tau) = 4 at the root.  Instead of
50 bisection iterations this kernel uses TWO probe evaluations plus a
model-based closed form:

  sigma0 = rowmax - 2                       (bracket lower end, f0 >= 4)
  eval0 -> (f0, g0), g = sum relu(z-sigma)  (g = -f'/2, exact)
  sigma1 = sigma0 + lam * phi-Newton step,  lam = c0 + c1(1 - 2/sqrt(f0))
          (phi-Newton = the beta=2 power-law step; lam over-relaxes it)
  eval1 -> (f1, g1)
  beta  = fit of f ~ A(c-sigma)^beta from the two log-derivatives
  n_hat = local active-lane count implied by the power model at sigma1
  tau   = root of the active-set quadratic n t^2 - 2 S1 t + (S2-4) = 0
          (exact while the active set is frozen), clamped to the bracket
  f_pred= quadratic model value at tau; p = relu(z-tau)^2 / f_pred
          (final normalization folded into the last pass as a
           per-partition scale u = 1/sqrt(f_pred))

Inputs are sent to the device as fp16 (host folds the mask; -30 stands in
for -inf), halving input DMA; all reductions/statistics stay fp32.
Verified vs the jax reference: norm_rel ~2.8e-3 (fit constants trained on
a synthetic batch from the same distribution, validated on real inputs).

Engine layout per [128, 4096] tile: DVE does rowmax + the two
max-accumulate probes + the final relu (4x fp16 mode); ScalarE does the
Square+accumulate legs and the final scaled Square (p written fp16, cast
to f32 during the output DMA on the SWDGE path).  Per-row statistics are
batched in [128, 8] tiles, two groups per core.
"""

import numpy as np

P = 128          # SBUF partitions
S = 4096         # row length
B_FULL = 16384   # total rows
N_CORES = 8
BP = B_FULL // N_CORES   # rows per core
NT = BP // P             # 16 tiles of 128 rows per core
G = 8                    # tiles per stat group
NG = NT // G             # stat groups per core

NEG = -30.0              # mask stand-in for -inf (buried below any sigma)
TARGET = 4.0             # 1/(alpha-1)^2 for alpha=1.5
C0L = 1.2                # lam = C0L + C1L*(1 - 2/sqrt(f0))
C1L = 0.6
NSCALE = 1.1             # n_hat scale
CLAMP_HI = 0.0312        # sigma* <= m - 2*(1/S)^0.5 = m - 0.03125

# per-group tile counts whose eval runs on ScalarE (Act relu+square pair)
# instead of DVE tensor_scalar + Act square -- load-balance knobs.
E0A = (3, 3)   # per-group: eval0 tiles running as Act relu+square pairs
E1A = (2, 0)   # per-group: eval1 Act-pair tiles (0 for last group: no DVE cover)
FDV = 5        # final-square legs on DVE (STT), last group only
# cast p fp16->f32 during the output DMA (SWDGE); False = f32 p + sync DMA
P_CAST_DMA = True
DEBUG_STATS = False

_CACHE = {}


def _build_program():
    import concourse.bacc as bacc
    import concourse.tile as tile
    import concourse.mybir as mybir
    from contextlib import ExitStack

    f32 = mybir.dt.float32
    f16 = mybir.dt.float16
    Alu = mybir.AluOpType
    Act = mybir.ActivationFunctionType
    X = mybir.AxisListType.X

    nc = bacc.Bacc(
        "TRN2",
        target_bir_lowering=False,
        debug=False,
        enable_asserts=False,
        num_devices=N_CORES,
    )
    z_d = nc.dram_tensor("z16", [BP, S], f16, kind="ExternalInput").ap()
    out_d = nc.dram_tensor("out", [BP, S], f32, kind="ExternalOutput").ap()
    dbg_d = None
    if DEBUG_STATS:
        dbg_d = nc.dram_tensor("dbg", [P, NG * 9 * G], f32, kind="ExternalOutput").ap()

    with tile.TileContext(nc) as tc, ExitStack() as ctx:
        z_pool = ctx.enter_context(tc.tile_pool(name="z", bufs=NT + 1))
        w_pool = ctx.enter_context(tc.tile_pool(name="w", bufs=3))
        f_pool = ctx.enter_context(tc.tile_pool(name="fld", bufs=2))
        p_pool = ctx.enter_context(tc.tile_pool(name="p", bufs=3))
        s_pool = ctx.enter_context(tc.tile_pool(name="st", bufs=2))
        c_pool = ctx.enter_context(tc.tile_pool(name="cst", bufs=1))

        def st(name, gi, dt=f32, w=G):
            return s_pool.tile([P, w], dt, tag=name, name=f"{name}_{gi}")

        # per-column S-vector: S for DVE-accum columns (gsum), 0 for
        # Act-accum columns (g direct).  Column t in a group corresponds
        # to tile gi*G + t; Act tiles are the first E*A columns.
        sv0g, sv1g = [], []
        act0sets, act1sets = [], []
        for gi in range(NG):
            sv0 = c_pool.tile([P, G], f32, tag=f"sv0_{gi}", name=f"sv0_{gi}")
            sv1 = c_pool.tile([P, G], f32, tag=f"sv1_{gi}", name=f"sv1_{gi}")
            a0set = {(i * G) // E0A[gi] + G // (2 * E0A[gi]) for i in range(E0A[gi])} if E0A[gi] else set()
            a1set = {(i * G) // E1A[gi] + G // (2 * E1A[gi]) for i in range(E1A[gi])} if E1A[gi] else set()
            for t in range(G):
                nc.vector.memset(sv0[:, t : t + 1], 0.0 if t in a0set else float(S))
                nc.vector.memset(sv1[:, t : t + 1], 0.0 if t in a1set else float(S))
            act0sets.append(a0set)
            act1sets.append(a1set)
            sv0g.append(sv0)
            sv1g.append(sv1)

        zs = [None] * NT
        grp = []
        for gi in range(NG):
            g = {}
            grp.append(g)
            g["M"] = st("M", gi)
            g["CH"] = st("CH", gi)
            g["s0"] = st("s0", gi)
            g["s0h"] = st("s0h", gi, f16)
            g["ns0"] = st("ns0", gi)
            g["acc0"] = st("acc0", gi)
            g["f0"] = st("f0", gi)
            g["g0"] = st("g0", gi)
            g["s1"] = st("s1", gi)
            g["s1h"] = st("s1h", gi, f16)
            g["ns1"] = st("ns1", gi)
            g["acc1"] = st("acc1", gi)
            g["f1"] = st("f1", gi)
            g["g1"] = st("g1", gi)
            g["tau"] = st("tau", gi)
            g["tauh"] = st("tauh", gi, f16)
            g["u"] = st("u", gi)
            g["u2"] = st("u2", gi)
            for t in ("t1", "t2", "t3", "t4", "t5"):
                g[t] = st(t, gi)

        def act_tiles(n):
            # n evenly spread tile indices in [0, G)
            return {(i * G) // n + (G // (2 * n) if n > 0 else 0) for i in range(n)} if n else set()

        # ---- stage 1: load + rowmax + sigma0 -------------------------
        def stage1(gi, lo=0, hi=G):
            g = grp[gi]
            for t in range(lo, hi):
                ti = gi * G + t
                row0 = ti * P
                z_t = z_pool.tile([P, S], f16, tag="z", name=f"z_{ti}")
                nc.sync.dma_start(z_t[:], z_d[row0 : row0 + P, :])
                zs[ti] = z_t
                fa = f_pool.tile([P, S // 2], f16, tag="fa", name=f"fa_{ti}")
                fb = f_pool.tile([P, S // 4], f16, tag="fb", name=f"fb_{ti}")
                nc.vector.tensor_tensor(
                    out=fa[:], in0=z_t[:, 0 : S // 2], in1=z_t[:, S // 2 : S],
                    op=Alu.max,
                )
                nc.vector.tensor_tensor(
                    out=fb[:], in0=fa[:, 0 : S // 4], in1=fa[:, S // 4 : S // 2],
                    op=Alu.max,
                )
                fc = f_pool.tile([P, S // 8], f16, tag="fc", name=f"fc_{ti}")
                nc.vector.tensor_tensor(
                    out=fc[:], in0=fb[:, 0 : S // 8], in1=fb[:, S // 8 : S // 4],
                    op=Alu.max,
                )
                nc.vector.reduce_max(g["M"][:, t : t + 1], fc[:], axis=X)
            # sigma0 = M - 2 (fp16 roundtrip so device-side arithmetic
            # sees exactly the value used by the tensor_scalar probes)
            c = slice(lo, hi)
            nc.vector.tensor_scalar(
                out=g["CH"][:, c], in0=g["M"][:, c], scalar1=-CLAMP_HI,
                scalar2=None, op0=Alu.add,
            )
            nc.vector.tensor_scalar(
                out=g["t1"][:, c], in0=g["M"][:, c], scalar1=-2.0, scalar2=None,
                op0=Alu.add,
            )
            nc.vector.tensor_copy(g["s0h"][:, c], g["t1"][:, c])
            nc.vector.tensor_copy(g["s0"][:, c], g["s0h"][:, c])
            if E0A[gi] > 0 or E1A[gi] > 0:
                nc.vector.tensor_scalar(
                    out=g["ns0"][:, c], in0=g["s0"][:, c], scalar1=-1.0,
                    scalar2=None, op0=Alu.mult,
                )

        # ---- stage 2: eval0 ------------------------------------------
        def stage2(gi, lo=0, hi=G):
            g = grp[gi]
            for t in range(lo, hi):
                ti = gi * G + t
                acc = g["acc0"][:, t : t + 1]
                fcol = g["f0"][:, t : t + 1]
                if t in act0sets[gi]:
                    r_t = w_pool.tile([P, S], f16, tag="w", name=f"r0_{ti}")
                    nc.scalar.activation(
                        r_t[:], zs[ti][:], Act.Relu,
                        bias=g["ns0"][:, t : t + 1], accum_out=acc,
                    )
                    nc.scalar.activation(
                        r_t[:], r_t[:], Act.Square, accum_out=fcol,
                    )
                else:
                    w_t = w_pool.tile([P, S], f16, tag="w", name=f"w0_{ti}")
                    nc.vector.tensor_scalar(
                        out=w_t[:], in0=zs[ti][:],
                        scalar1=g["s0"][:, t : t + 1], scalar2=None,
                        op0=Alu.max, op1=Alu.add, accum_out=acc,
                    )
                    # (sigma0 - mx)^2 = relu(z-sigma0)^2
                    nc.scalar.activation(
                        w_t[:], w_t[:], Act.Square,
                        bias=g["s0"][:, t : t + 1], scale=-1.0, accum_out=fcol,
                    )

        # ---- stage 3: stats B: sigma1 --------------------------------
        def stage3a(gi):
            g = grp[gi]
            # g0 = acc0 - Svec*sigma0 (Act columns accumulated g directly)
            nc.vector.tensor_tensor(out=g["t1"][:], in0=g["s0"][:], in1=sv0g[gi][:], op=Alu.mult)
            nc.vector.tensor_tensor(out=g["g0"][:], in0=g["acc0"][:], in1=g["t1"][:], op=Alu.subtract)
            # sq0 = sqrt(f0) issued on ScalarE; rest continues in stage3b
            nc.scalar.activation(g["t2"][:], g["f0"][:], Act.Sqrt)
            nc.vector.reciprocal(g["t3"][:], g["g0"][:])

        def stage3b(gi):
            g = grp[gi]
            nc.vector.scalar_tensor_tensor(
                out=g["t4"][:], in0=g["t2"][:], scalar=-2.0, in1=g["f0"][:],
                op0=Alu.mult, op1=Alu.add,
            )
            nc.vector.tensor_tensor(out=g["t4"][:], in0=g["t4"][:], in1=g["t3"][:], op=Alu.mult)
            # lam = (C0L+C1L) - 2*C1L/sq0
            nc.vector.reciprocal(g["t5"][:], g["t2"][:])
            nc.vector.tensor_scalar(
                out=g["t5"][:], in0=g["t5"][:], scalar1=-2.0 * C1L,
                scalar2=C0L + C1L, op0=Alu.mult, op1=Alu.add,
            )
            nc.vector.tensor_tensor(out=g["t4"][:], in0=g["t4"][:], in1=g["t5"][:], op=Alu.mult)
            nc.vector.tensor_tensor(out=g["s1"][:], in0=g["t4"][:], in1=g["s0"][:], op=Alu.add)
            nc.vector.tensor_tensor(out=g["s1"][:], in0=g["s1"][:], in1=g["CH"][:], op=Alu.min)
            nc.vector.tensor_tensor(out=g["s1"][:], in0=g["s1"][:], in1=g["s0"][:], op=Alu.max)
            nc.vector.tensor_copy(g["s1h"][:], g["s1"][:])
            nc.vector.tensor_copy(g["s1"][:], g["s1h"][:])
            if E1A[gi] > 0:
                nc.vector.tensor_scalar(
                    out=g["ns1"][:], in0=g["s1"][:], scalar1=-1.0, scalar2=None,
                    op0=Alu.mult,
                )

        # ---- stage 4: eval1 ------------------------------------------
        def stage4(gi):
            g = grp[gi]
            for t in range(G):
                ti = gi * G + t
                acc = g["acc1"][:, t : t + 1]
                fcol = g["f1"][:, t : t + 1]
                if t in act1sets[gi]:
                    r_t = w_pool.tile([P, S], f16, tag="w", name=f"r1_{ti}")
                    nc.scalar.activation(
                        r_t[:], zs[ti][:], Act.Relu,
                        bias=g["ns1"][:, t : t + 1], accum_out=acc,
                    )
                    nc.scalar.activation(
                        r_t[:], r_t[:], Act.Square, accum_out=fcol,
                    )
                else:
                    w_t = w_pool.tile([P, S], f16, tag="w", name=f"w1_{ti}")
                    nc.vector.tensor_scalar(
                        out=w_t[:], in0=zs[ti][:],
                        scalar1=g["s1"][:, t : t + 1], scalar2=None,
                        op0=Alu.max, op1=Alu.add, accum_out=acc,
                    )
                    nc.scalar.activation(
                        w_t[:], w_t[:], Act.Square,
                        bias=g["s1"][:, t : t + 1], scale=-1.0, accum_out=fcol,
                    )

        # ---- stage 5: stats C: tau, u --------------------------------
        def stage5a(gi):
            g = grp[gi]
            nc.vector.tensor_tensor(out=g["t1"][:], in0=g["s1"][:], in1=sv1g[gi][:], op=Alu.mult)
            nc.vector.tensor_tensor(out=g["g1"][:], in0=g["acc1"][:], in1=g["t1"][:], op=Alu.subtract)
            # a0 = 2 g0/f0, a1 = 2 g1/f1
            nc.vector.reciprocal(g["t1"][:], g["f0"][:])
            nc.vector.scalar_tensor_tensor(
                out=g["t1"][:], in0=g["g0"][:], scalar=2.0, in1=g["t1"][:],
                op0=Alu.mult, op1=Alu.mult,
            )
            nc.vector.reciprocal(g["t2"][:], g["f1"][:])
            nc.vector.scalar_tensor_tensor(
                out=g["t2"][:], in0=g["g1"][:], scalar=2.0, in1=g["t2"][:],
                op0=Alu.mult, op1=Alu.mult,
            )
            # beta = (s1-s0) a0 a1 / max(a1-a0, 1e-8), clamped [2, 64]
            nc.vector.tensor_tensor(out=g["t3"][:], in0=g["t2"][:], in1=g["t1"][:], op=Alu.subtract)
            nc.vector.tensor_scalar(
                out=g["t3"][:], in0=g["t3"][:], scalar1=1e-8, scalar2=None, op0=Alu.max,
            )
            nc.vector.reciprocal(g["t3"][:], g["t3"][:])
            nc.vector.tensor_tensor(out=g["t4"][:], in0=g["s1"][:], in1=g["s0"][:], op=Alu.subtract)
            nc.vector.tensor_tensor(out=g["t4"][:], in0=g["t4"][:], in1=g["t1"][:], op=Alu.mult)
            nc.vector.tensor_tensor(out=g["t4"][:], in0=g["t4"][:], in1=g["t2"][:], op=Alu.mult)
            nc.vector.tensor_tensor(out=g["t4"][:], in0=g["t4"][:], in1=g["t3"][:], op=Alu.mult)
            nc.vector.tensor_scalar(
                out=g["t4"][:], in0=g["t4"][:], scalar1=64.0, scalar2=2.0,
                op0=Alu.min, op1=Alu.max,
            )
            # n_hat = max(NSCALE g1 a1 (beta-1)/beta, 1)
            nc.vector.reciprocal(g["t3"][:], g["t4"][:])
            nc.vector.tensor_scalar(
                out=g["t4"][:], in0=g["t4"][:], scalar1=-1.0, scalar2=None, op0=Alu.add,
            )
            nc.vector.tensor_tensor(out=g["t4"][:], in0=g["t4"][:], in1=g["t3"][:], op=Alu.mult)
            nc.vector.scalar_tensor_tensor(
                out=g["t3"][:], in0=g["g1"][:], scalar=NSCALE, in1=g["t2"][:],
                op0=Alu.mult, op1=Alu.mult,
            )
            nc.vector.tensor_tensor(out=g["t3"][:], in0=g["t3"][:], in1=g["t4"][:], op=Alu.mult)
            nc.vector.tensor_scalar(
                out=g["t3"][:], in0=g["t3"][:], scalar1=1.0, scalar2=None, op0=Alu.max,
            )
            # t3 = n_hat.  S1 = g1 + n s1 -> t4 ; S2 = f1 + 2 s1 S1 - n s1^2 -> t5
            nc.vector.tensor_tensor(out=g["t1"][:], in0=g["t3"][:], in1=g["s1"][:], op=Alu.mult)
            nc.vector.tensor_tensor(out=g["t4"][:], in0=g["t1"][:], in1=g["g1"][:], op=Alu.add)
            nc.vector.tensor_tensor(out=g["t2"][:], in0=g["t4"][:], in1=g["s1"][:], op=Alu.mult)
            nc.vector.tensor_tensor(out=g["t1"][:], in0=g["t1"][:], in1=g["s1"][:], op=Alu.mult)
            nc.vector.scalar_tensor_tensor(
                out=g["t2"][:], in0=g["t2"][:], scalar=2.0, in1=g["f1"][:],
                op0=Alu.mult, op1=Alu.add,
            )
            nc.vector.tensor_tensor(out=g["t5"][:], in0=g["t2"][:], in1=g["t1"][:], op=Alu.subtract)
            # disc = S1^2 - n (S2 - 4) -> t2 ; tau = (S1 - sqrt(disc))/n
            nc.vector.tensor_scalar(
                out=g["t1"][:], in0=g["t5"][:], scalar1=-TARGET, scalar2=None, op0=Alu.add,
            )
            nc.vector.tensor_tensor(out=g["t1"][:], in0=g["t1"][:], in1=g["t3"][:], op=Alu.mult)
            nc.vector.tensor_tensor(out=g["t2"][:], in0=g["t4"][:], in1=g["t4"][:], op=Alu.mult)
            nc.vector.tensor_tensor(out=g["t2"][:], in0=g["t2"][:], in1=g["t1"][:], op=Alu.subtract)
            nc.vector.tensor_scalar(
                out=g["t2"][:], in0=g["t2"][:], scalar1=0.0, scalar2=None, op0=Alu.max,
            )
            nc.scalar.activation(g["t2"][:], g["t2"][:], Act.Sqrt)
            nc.vector.reciprocal(g["t1"][:], g["t3"][:])

        def stage5b(gi, fdv):
            g = grp[gi]
            # tau_pre = (S1 - sqrt(disc))/n -> t2 (kept unclamped for u)
            nc.vector.tensor_tensor(out=g["t2"][:], in0=g["t4"][:], in1=g["t2"][:], op=Alu.subtract)
            nc.vector.tensor_tensor(out=g["t2"][:], in0=g["t2"][:], in1=g["t1"][:], op=Alu.mult)
            nc.vector.tensor_tensor(out=g["tau"][:], in0=g["t2"][:], in1=g["CH"][:], op=Alu.min)
            nc.vector.tensor_tensor(out=g["tau"][:], in0=g["tau"][:], in1=g["s0"][:], op=Alu.max)
            nc.vector.tensor_copy(g["tauh"][:], g["tau"][:])
            nc.vector.tensor_copy(g["tau"][:], g["tauh"][:])
            # first-order u around the model root: e = f_pred - 4 = -2 gm delta,
            # u = 0.5 - e/16 + (3/256) e^2
            nc.vector.tensor_tensor(out=g["t1"][:], in0=g["t2"][:], in1=g["tau"][:], op=Alu.subtract)
            nc.vector.tensor_tensor(out=g["t5"][:], in0=g["t3"][:], in1=g["tau"][:], op=Alu.mult)
            nc.vector.tensor_tensor(out=g["t5"][:], in0=g["t4"][:], in1=g["t5"][:], op=Alu.subtract)
            nc.vector.scalar_tensor_tensor(
                out=g["t1"][:], in0=g["t5"][:], scalar=-2.0, in1=g["t1"][:],
                op0=Alu.mult, op1=Alu.mult,
            )
            nc.vector.tensor_tensor(out=g["t2"][:], in0=g["t1"][:], in1=g["t1"][:], op=Alu.mult)
            nc.vector.tensor_scalar(
                out=g["t2"][:], in0=g["t2"][:], scalar1=3.0 / 256.0, scalar2=0.5,
                op0=Alu.mult, op1=Alu.add,
            )
            nc.vector.scalar_tensor_tensor(
                out=g["u"][:], in0=g["t1"][:], scalar=-1.0 / 16.0, in1=g["t2"][:],
                op0=Alu.mult, op1=Alu.add,
            )
            if fdv > 0:
                nc.vector.tensor_tensor(out=g["u2"][:], in0=g["u"][:], in1=g["u"][:], op=Alu.mult)

        if DEBUG_STATS:
            for gi in range(NG):
                g = grp[gi]
                names = ["M", "s0", "f0", "g0", "s1", "f1", "g1", "tau", "u"]
                for k, nm in enumerate(names):
                    off = (gi * 9 + k) * G
                    nc.sync.dma_start(dbg_d[:, off : off + G], g[nm][:])

        # ---- stage 6: final pass + store -----------------------------
        def stage6(gi, fdv):
            g = grp[gi]
            for t in range(G):
                ti = gi * G + t
                row0 = ti * P
                v_t = w_pool.tile([P, S], f16, tag="w", name=f"v_{ti}")
                nc.vector.tensor_scalar(
                    out=v_t[:], in0=zs[ti][:],
                    scalar1=g["tau"][:, t : t + 1], scalar2=g["tau"][:, t : t + 1],
                    op0=Alu.max, op1=Alu.subtract,
                )
                p_t = p_pool.tile([P, S], f16, tag="p", name=f"p_{ti}")
                if t >= G - fdv:
                    # p = (s*v)*v on DVE (s = u^2 = 1/f_pred), fp16 2x
                    nc.vector.scalar_tensor_tensor(
                        out=p_t[:], in0=v_t[:], scalar=g["u2"][:, t : t + 1],
                        in1=v_t[:], op0=Alu.mult, op1=Alu.mult,
                    )
                else:
                    nc.scalar.activation(
                        p_t[:], v_t[:], Act.Square, scale=g["u"][:, t : t + 1],
                    )
                nc.gpsimd.dma_start(out_d[row0 : row0 + P, :], p_t[:])

        # ---- software-pipelined issue order --------------------------
        if NG == 2:
            stage1(0, 0, 4); stage2(0, 0, 4); stage1(0, 4, 8); stage2(0, 4, 8)
            stage3a(0); stage1(1); stage3b(0); stage4(0); stage5a(0)
            stage2(1, 0, 4); stage5b(0, 0); stage6(0, 0); stage2(1, 4, 8)
            stage3a(1); stage3b(1); stage4(1); stage5a(1); stage5b(1, FDV)
            stage6(1, FDV)
        else:
            for gi in range(NG):
                stage1(gi)
            for gi in range(NG):
                stage2(gi); stage3a(gi); stage3b(gi); stage4(gi)
                stage5a(gi); stage5b(gi, FDV if gi == NG - 1 else 0)
            for gi in range(NG):
                stage6(gi, FDV if gi == NG - 1 else 0)


    nc.compile()
    return nc


def _get_program():
    if "nc" not in _CACHE:
        _CACHE["nc"] = _build_program()
    return _CACHE["nc"]


def _prep_z16(scores, mask_b):
    z16 = scores.astype(np.float16)
    np.copyto(z16, np.float16(NEG), where=~mask_b)
    return np.ascontiguousarray(z16)


def _kernel_numpy_fallback(scores, mask, alpha):
    """Reference-equivalent host computation (only for alpha != 1.5)."""
    f32 = np.float32
    alpha = max(float(alpha), 1.0)
    am1 = alpha - 1.0
    x = np.where(mask, scores, f32(-1e9)).astype(f32)
    Xs = (x * f32(am1)).astype(f32)
    mx = Xs.max(axis=-1, keepdims=True)
    tau_lo = mx - f32(1.0)
    tau_hi = mx - f32((1.0 / x.shape[-1]) ** am1)
    dm = tau_hi - tau_lo
    tau_m = tau_lo
    inv = f32(1.0 / am1)
    for _ in range(50):
        dm = dm / 2
        tau_m = tau_lo + dm
        p = np.clip(Xs - tau_m, 0.0, None) ** inv
        f = p.sum(axis=-1, keepdims=True) - 1.0
        tau_lo = np.where(f >= 0, tau_m, tau_lo)
    p = np.clip(Xs - tau_m, 0.0, None) ** inv
    return (p / p.sum(axis=-1, keepdims=True)).astype(f32)


def kernel(scores, mask, alpha):
    scores = np.asarray(scores, dtype=np.float32)
    mask_b = np.asarray(mask).astype(bool)
    alpha_v = float(np.asarray(alpha))

    if abs(max(alpha_v, 1.0) - 1.5) > 1e-6:
        return _kernel_numpy_fallback(scores, mask_b, alpha_v)

    z16 = _prep_z16(scores, mask_b)

    from concourse import bass_utils

    nc = _get_program()
    in_maps = [{"z16": z16[i * BP : (i + 1) * BP]} for i in range(N_CORES)]
    res = bass_utils.run_bass_kernel_spmd(nc, in_maps, core_ids=list(range(N_CORES)))
    return np.concatenate([r["out"] for r in res.results], axis=0)
